# revision 8
# baseline (speedup 1.0000x reference)
"""Trainium2 Bass kernel for nn_DeepGcnV2 (GCNII-style message passing).

Data-parallel over the batch of 32 graphs: 4 graphs per NeuronCore on 8
cores.

Math: per layer  z = relu(LN(h));  s = (1-a)*Ahat@z + a*z;  h += s @ Wt
with Ahat = D^-1/2 (A+I) D^-1/2 and Wt = (1-beta) I + beta W.

Key restructuring vs a direct port: the whole normalized propagation
matrix is folded on the HOST into a single bf16 operand

    B = 0.9 * Ahat + 0.1 * I          (shipped as B^T, j-major rows)

so the device per layer does just two dense matmul groups and zero
transposes:

    s^T = z^T @ B^T     (lhsT = z node-major tiles, rhs = B^T streaming,
                         psum out is feature-major [H, N])
    h  += s @ Wt        (lhsT = s^T slices, rhs = Wt -> node-major psum)

LN statistics are free-dim reductions in node-major layout.  Degree
computation, dinv scalings, and the GCNII residual coefficients all
live inside B, so the device does no per-node elementwise scaling.
"""

import sys

for _p in ("/opt/trn_rl_repo",):
    if _p not in sys.path:
        sys.path.insert(0, _p)

import math

import ml_dtypes
import numpy as np

import concourse.bacc as bacc
import concourse.mybir as mybir
import concourse.tile as tile
from concourse.bass_types import AP
from concourse.bass_utils import run_bass_kernel_spmd

# Problem constants (hardcoded per contract)
BS, N, D, H, L = 32, 2048, 128, 64, 4
ALPHA, LAMDA, EPS = 0.1, 1.0, 1e-5
NCORES = 8
G = BS // NCORES          # graphs per core
P = 128                   # partitions
NT = N // P               # node tiles per graph
HALF = NT // 2            # tiles per psum half-batch
NCH = 4                   # aggregation psum chunks
CHW = N // NCH            # chunk width (512)

f32 = mybir.dt.float32
bf16 = mybir.dt.bfloat16


def _bmid(ap, n):
    """[128, F] AP -> [128, n, F] AP broadcast along an inserted middle dim."""
    dims = list(map(list, ap.ap))
    assert len(dims) == 2, dims
    return AP(ap.tensor, ap.offset, [dims[0], [0, n], dims[1]])


def _build_nc(trivial_affine=True, reps=1, graphs=G, skip_agg=False, skip_w=False):
    nc = bacc.Bacc("TRN2", target_bir_lowering=False, debug=False)

    bt = nc.dram_tensor("bt", [G, N, N], bf16, kind="ExternalInput").ap()
    xt = nc.dram_tensor("xt", [G, D, N], bf16, kind="ExternalInput").ap()
    # pk slot 0: proj_w [128, H]; slot 1+l: Wt[l] in rows 0:64
    pk = nc.dram_tensor("pk", [P, 1 + L, H], bf16, kind="ExternalInput").ap()
    hwv = nc.dram_tensor("hwv", [P, H], f32, kind="ExternalInput").ap()
    if not trivial_affine:
        # slots: 0..L-1 ln_g, L..2L-1 ln_b, 2L proj_b, 2L+1 head_b(col 0)
        aux = nc.dram_tensor("aux", [P, 2 * L + 2, H], f32, kind="ExternalInput").ap()
    out = nc.dram_tensor("out", [G, N, 1], f32, kind="ExternalOutput").ap()

    AX = mybir.AxisListType.X
    ADD = mybir.AluOpType.add
    MUL = mybir.AluOpType.mult
    SUB = mybir.AluOpType.subtract
    AF = mybir.ActivationFunctionType

    from contextlib import ExitStack
    with tile.TileContext(nc) as tc, ExitStack() as ctx:
        ep = ctx.enter_context
        cpool = ep(tc.tile_pool(name="const", bufs=1))
        prm = ep(tc.tile_pool(name="prm", bufs=1))
        bt_pool = ep(tc.tile_pool(name="bt", bufs=2))
        xt_pool = ep(tc.tile_pool(name="xts", bufs=2))
        h_pool = ep(tc.tile_pool(name="h", bufs=2))
        z_pool = ep(tc.tile_pool(name="z", bufs=2))
        st_pool = ep(tc.tile_pool(name="st", bufs=2))
        big_pool = ep(tc.tile_pool(name="big", bufs=2))
        sm_pool = ep(tc.tile_pool(name="sm", bufs=4))
        ob_pool = ep(tc.tile_pool(name="ob", bufs=2))
        ps_s = ep(tc.tile_pool(name="ps_s", bufs=2, space="PSUM"))
        ps_w = ep(tc.tile_pool(name="ps_w", bufs=2, space="PSUM"))

        eps_sb = cpool.tile([P, 1], f32)
        nc.vector.memset(eps_sb[:], EPS)

        pk_sb = prm.tile([P, 1 + L, H], bf16)
        nc.sync.dma_start(pk_sb[:], pk)
        hwv_sb = prm.tile([P, H], f32)
        nc.sync.dma_start(hwv_sb[:], hwv)
        if not trivial_affine:
            aux_sb = prm.tile([P, 2 * L + 2, H], f32)
            nc.sync.dma_start(aux_sb[:], aux)

        for g in [gg for _ in range(reps) for gg in range(graphs)]:
            # ---- load B^T (bf16) and x^T (bf16) ----
            bt_sb = bt_pool.tile([P, NT, N], bf16)
            nc.sync.dma_start(
                bt_sb[:], bt[g].rearrange("(jb p) i -> p jb i", p=P))
            xt_sb = xt_pool.tile([D, N], bf16)
            nc.sync.dma_start(xt_sb[:], xt[g])

            # ---- h0 = x @ proj_w (+ proj_b) ----
            h = h_pool.tile([P, NT, H], f32)
            for half in range(2):
                hp = ps_w.tile([P, HALF, H], f32)
                for q in range(HALF):
                    i = half * HALF + q
                    nc.tensor.matmul(
                        hp[:, q, :], lhsT=xt_sb[:, i * P:(i + 1) * P],
                        rhs=pk_sb[:, 0, :], start=True, stop=True)
                hs = slice(half * HALF, (half + 1) * HALF)
                if trivial_affine:
                    nc.vector.tensor_copy(h[:, hs, :], hp[:, :, :])
                else:
                    nc.vector.tensor_tensor(
                        h[:, hs, :], hp[:, :, :],
                        _bmid(aux_sb[:, 2 * L, :], HALF), op=ADD)

            # ---- layers ----
            for l in range(L):
                # LN stats: mu, rstd per node
                musum = sm_pool.tile([P, NT], f32, tag="musum")
                nc.vector.tensor_reduce(musum[:], h[:, :, :], axis=AX, op=ADD)
                sqh = big_pool.tile([P, NT, H], f32, tag="big")
                nc.scalar.activation(sqh[:], h[:, :, :], AF.Square)
                ssq = sm_pool.tile([P, NT], f32, tag="ssq")
                nc.vector.tensor_reduce(ssq[:], sqh[:, :, :], axis=AX, op=ADD)
                mu = sm_pool.tile([P, NT], f32, tag="mu")
                nc.vector.tensor_scalar(mu[:], musum[:], 1.0 / H, None, MUL)
                var = sm_pool.tile([P, NT], f32, tag="var")
                nc.vector.tensor_tensor(var[:], mu[:], mu[:], op=MUL)
                nc.vector.tensor_scalar(ssq[:], ssq[:], 1.0 / H, None, MUL)
                nc.vector.tensor_tensor(var[:], ssq[:], var[:], op=SUB)
                stdv = sm_pool.tile([P, NT], f32, tag="stdv")
                nc.scalar.activation(stdv[:], var[:], AF.Sqrt, bias=eps_sb[:, 0:1])
                rstd = sm_pool.tile([P, NT], f32, tag="rstd")
                nc.vector.reciprocal(rstd[:], stdv[:])

                # z = relu((h - mu) * rstd [* g + b])  (bf16, node-major)
                zf = big_pool.tile([P, NT, H], f32, tag="big")
                nc.vector.tensor_tensor(
                    zf[:, :, :], h[:, :, :], mu[:].broadcast_to([P, NT, H]), op=SUB)
                nc.vector.tensor_tensor(
                    zf[:, :, :], zf[:, :, :], rstd[:].broadcast_to([P, NT, H]), op=MUL)
                if not trivial_affine:
                    nc.vector.tensor_tensor(
                        zf[:, :, :], zf[:, :, :], _bmid(aux_sb[:, l, :], NT), op=MUL)
                    nc.vector.tensor_tensor(
                        zf[:, :, :], zf[:, :, :], _bmid(aux_sb[:, L + l, :], NT), op=ADD)
                z = z_pool.tile([P, NT, H], bf16)
                nc.scalar.activation(z[:, :, :], zf[:, :, :], AF.Relu)

                # s^T = z^T @ B^T, col-tiled 2x: PE cols 0:63 (chain A) compute
                # node-columns 0:1023 -> psum rows 0:64; cols 64:127 (chain B)
                # compute 1024:2047 -> psum rows 64:128.  Both chains share the
                # z weights and run concurrently.
                # st layout: [0:64, 0:1024] = s^T[:, 0:1024];
                #            [64:128, 0:1024] = s^T[:, 1024:2048]
                st = st_pool.tile([P, N // 2], bf16)
                if skip_agg:
                    nc.vector.memset(st[:, :], 0.0)
                else:
                    pss = ps_s.tile([P, N // 2], f32)
                    for cp in range(2):   # chunk pair: A chunk cp, B chunk 2+cp
                        csA = slice(cp * CHW, (cp + 1) * CHW)
                        csB = slice((2 + cp) * CHW, (3 + cp) * CHW)
                        csP = slice(cp * CHW, (cp + 1) * CHW)
                        for jb in range(NT):
                            nc.tensor.matmul(
                                pss[0:H, csP], lhsT=z[:, jb, :],
                                rhs=bt_sb[:, jb, csA],
                                start=(jb == 0), stop=(jb == NT - 1))
                            nc.tensor.matmul(
                                pss[H:P, csP], lhsT=z[:, jb, :],
                                rhs=bt_sb[:, jb, csB],
                                start=(jb == 0), stop=(jb == NT - 1))
                        # copy each half-chunk out as it completes
                        if cp == 0:
                            nc.vector.tensor_copy(st[0:H, csP], pss[0:H, csP])
                            nc.scalar.copy(st[H:P, csP], pss[H:P, csP])
                        else:
                            nc.scalar.copy(st[0:H, csP], pss[0:H, csP])
                            nc.vector.tensor_copy(st[H:P, csP], pss[H:P, csP])

                # h += s @ Wt[l]   (lhsT = s^T slices -> node-major psum);
                # node tiles i<8 read st rows 0:64 (PE rows 0:63), i>=8 read
                # rows 64:128 (PE rows 64:127) -> row-tiled concurrency.
                for half in range(0 if skip_w else 2):
                    wp = ps_w.tile([P, HALF, H], f32)
                    for q in range(HALF):
                        i = half * HALF + q
                        if i < NT // 2:
                            lhsT = st[0:H, i * P:(i + 1) * P]
                            rhs = pk_sb[0:H, 1 + l, :]
                        else:
                            j = i - NT // 2
                            lhsT = st[H:P, j * P:(j + 1) * P]
                            rhs = pk_sb[H:P, 1 + l, :]
                        nc.tensor.matmul(
                            wp[:, q, :], lhsT=lhsT, rhs=rhs,
                            start=True, stop=True)
                    hs = slice(half * HALF, (half + 1) * HALF)
                    nc.vector.tensor_tensor(
                        h[:, hs, :], h[:, hs, :], wp[:, :, :], op=ADD)

            # ---- head: out = h @ head_w (+ head_b) ----
            th = big_pool.tile([P, NT, H], f32, tag="big")
            nc.gpsimd.tensor_tensor(
                th[:, :, :], h[:, :, :], _bmid(hwv_sb[:, :], NT), op=MUL)
            osb = ob_pool.tile([P, NT], f32)
            nc.vector.tensor_reduce(osb[:], th[:, :, :], axis=AX, op=ADD)
            if not trivial_affine:
                nc.vector.tensor_scalar(
                    osb[:], osb[:], aux_sb[:, 2 * L + 1, 0:1], None, ADD)
            nc.sync.dma_start(
                out=out[g].rearrange("(ib p) one -> p (ib one)", p=P),
                in_=osb[:, :])

    nc.compile()
    return nc


_NC = {}


def _get_nc(trivial_affine=True):
    key = trivial_affine
    if key not in _NC:
        _NC[key] = _build_nc(trivial_affine=trivial_affine)
    return _NC[key]


def _prep_in_maps(inputs):
    x = np.asarray(inputs["x"], np.float32)
    adj = np.asarray(inputs["adj"], np.float32)
    proj_w = np.asarray(inputs["proj_w"], np.float32)
    proj_b = np.asarray(inputs["proj_b"], np.float32)
    ln_g = np.asarray(inputs["ln_g"], np.float32)
    ln_b = np.asarray(inputs["ln_b"], np.float32)
    conv_w = np.asarray(inputs["conv_w"], np.float32)
    head_w = np.asarray(inputs["head_w"], np.float32)
    head_b = np.asarray(inputs["head_b"], np.float32)

    trivial_affine = bool(
        np.all(ln_g == 1.0) and np.all(ln_b == 0.0)
        and np.all(proj_b == 0.0) and np.all(head_b == 0.0))

    # Wt[l] = (1-beta) I + beta conv_w[l], replicated in both partition
    # halves (row-tiled w-matmuls read rows 0:64 or 64:128)
    pkh = np.zeros((P, 1 + L, H), np.float32)
    pkh[:, 0, :] = proj_w
    for l in range(L):
        beta = math.log(LAMDA / (l + 1) + 1.0)
        wt = (1.0 - beta) * np.eye(H, dtype=np.float32) + beta * conv_w[l]
        pkh[:H, 1 + l, :] = wt
        pkh[H:, 1 + l, :] = wt

    # B^T per graph, bf16: bt[j, i] = 0.9*d_j*d_i*((A!=0)^T + I)[j,i] + 0.1*I
    ey = np.eye(N, dtype=np.float32)
    bt_all = np.empty((BS, N, N), ml_dtypes.bfloat16)
    sc = np.empty((N,), np.float32)
    for gi in range(BS):
        pat = adj[gi] != 0
        deg = pat.sum(-1, dtype=np.float32) + 1.0
        np.sqrt(0.9 / deg, out=sc)                      # sqrt(0.9) * deg^-1/2
        b = pat.T.astype(np.float32)
        b += ey
        b *= sc[:, None]
        b *= sc[None, :]
        b[np.arange(N), np.arange(N)] += 0.1
        bt_all[gi] = b.astype(ml_dtypes.bfloat16)

    xt_all = np.ascontiguousarray(
        x.transpose(0, 2, 1)).astype(ml_dtypes.bfloat16)

    shared = {
        "pk": pkh.astype(ml_dtypes.bfloat16),
        "hwv": np.ascontiguousarray(
            np.broadcast_to(head_w[:, 0][None, :], (P, H))),
    }
    if not trivial_affine:
        aux = np.zeros((P, 2 * L + 2, H), np.float32)
        aux[:, :L, :] = np.broadcast_to(ln_g[:, None, :], (L, P, H)).transpose(1, 0, 2)
        aux[:, L:2 * L, :] = np.broadcast_to(ln_b[:, None, :], (L, P, H)).transpose(1, 0, 2)
        aux[:, 2 * L, :] = proj_b[None, :]
        aux[:, 2 * L + 1, 0] = head_b[0]
        shared["aux"] = aux

    in_maps = []
    for c in range(NCORES):
        sl = slice(c * G, (c + 1) * G)
        in_maps.append(dict(
            shared,
            bt=np.ascontiguousarray(bt_all[sl]),
            xt=np.ascontiguousarray(xt_all[sl]),
        ))
    return in_maps, trivial_affine


def kernel(**inputs) -> np.ndarray:
    in_maps, trivial_affine = _prep_in_maps(inputs)
    nc = _get_nc(trivial_affine)
    res = run_bass_kernel_spmd(nc, in_maps, list(range(NCORES)))
    return np.concatenate([res.results[c]["out"] for c in range(NCORES)], axis=0)


# revision 16
# speedup vs baseline: 1.4562x; 1.4562x over previous
"""Trainium2 Bass kernel for nn_DeepGcnV2 (GCNII-style message passing).

Data-parallel over the batch of 32 graphs: 4 graphs per NeuronCore on 8
cores.

Math: per layer  z = relu(LN(h));  s = (1-a)*Ahat@z + a*z;  h += s @ Wt
with Ahat = D^-1/2 (A+I) D^-1/2 and Wt = (1-beta) I + beta W.

Key restructuring vs a direct port: the whole normalized propagation
matrix is folded on the HOST into a single bf16 operand

    B = 0.9 * Ahat + 0.1 * I          (shipped as B^T, j-major rows)

so the device per layer does just two dense matmul groups and zero
transposes:

    s^T = z^T @ B^T     (lhsT = z node-major tiles, rhs = B^T streaming,
                         psum out is feature-major [H, N])
    h  += s @ Wt        (lhsT = s^T slices, rhs = Wt -> node-major psum)

LN statistics are free-dim reductions in node-major layout.  Degree
computation, dinv scalings, and the GCNII residual coefficients all
live inside B, so the device does no per-node elementwise scaling.
"""

import sys

for _p in ("/opt/trn_rl_repo",):
    if _p not in sys.path:
        sys.path.insert(0, _p)

import math

import ml_dtypes
import numpy as np

import concourse.bacc as bacc
import concourse.mybir as mybir
import concourse.tile as tile
from concourse.bass_types import AP
from concourse.bass_utils import run_bass_kernel_spmd

# Problem constants (hardcoded per contract)
BS, N, D, H, L = 32, 2048, 128, 64, 4
ALPHA, LAMDA, EPS = 0.1, 1.0, 1e-5
NCORES = 8
G = BS // NCORES          # graphs per core
P = 128                   # partitions
NT = N // P               # node tiles per graph
HALF = NT // 2            # tiles per psum half-batch
NCH = 4                   # aggregation psum chunks
CHW = N // NCH            # chunk width (512)

f32 = mybir.dt.float32
bf16 = mybir.dt.bfloat16


def _bmid(ap, n):
    """[128, F] AP -> [128, n, F] AP broadcast along an inserted middle dim."""
    dims = list(map(list, ap.ap))
    assert len(dims) == 2, dims
    return AP(ap.tensor, ap.offset, [dims[0], [0, n], dims[1]])


def _build_nc(trivial_affine=True, reps=1, graphs=G, skip_agg=False, skip_w=False,
              col_tile=True):
    nc = bacc.Bacc("TRN2", target_bir_lowering=False, debug=False)

    bt = nc.dram_tensor("bt", [G, N, N], bf16, kind="ExternalInput").ap()
    xt = nc.dram_tensor("xt", [G, D, N], bf16, kind="ExternalInput").ap()
    # pk slot 0: proj_w [128, H]; slot 1+l: Wt[l] in rows 0:64
    pk = nc.dram_tensor("pk", [P, 1 + L, H], bf16, kind="ExternalInput").ap()
    hwv = nc.dram_tensor("hwv", [P, H], f32, kind="ExternalInput").ap()
    if not trivial_affine:
        # slots: 0..L-1 ln_g, L..2L-1 ln_b, 2L proj_b, 2L+1 head_b(col 0)
        aux = nc.dram_tensor("aux", [P, 2 * L + 2, H], f32, kind="ExternalInput").ap()
    out = nc.dram_tensor("out", [G, N, 1], f32, kind="ExternalOutput").ap()

    AX = mybir.AxisListType.X
    ADD = mybir.AluOpType.add
    MUL = mybir.AluOpType.mult
    SUB = mybir.AluOpType.subtract
    AF = mybir.ActivationFunctionType

    from contextlib import ExitStack
    with tile.TileContext(nc) as tc, ExitStack() as ctx:
        ep = ctx.enter_context
        cpool = ep(tc.tile_pool(name="const", bufs=1))
        prm = ep(tc.tile_pool(name="prm", bufs=1))
        bt_pool = ep(tc.tile_pool(name="bt", bufs=2))
        xt_pool = ep(tc.tile_pool(name="xts", bufs=2))
        h_pool = ep(tc.tile_pool(name="h", bufs=2))
        z_pool = ep(tc.tile_pool(name="z", bufs=2))
        st_pool = ep(tc.tile_pool(name="st", bufs=2))
        big_pool = ep(tc.tile_pool(name="big", bufs=2))
        sm_pool = ep(tc.tile_pool(name="sm", bufs=4))
        ob_pool = ep(tc.tile_pool(name="ob", bufs=2))
        ps_s = ep(tc.tile_pool(name="ps_s", bufs=2, space="PSUM"))
        ps_w = ep(tc.tile_pool(name="ps_w", bufs=2, space="PSUM"))

        eps_sb = cpool.tile([P, 1], f32)
        nc.vector.memset(eps_sb[:], EPS)

        pk_sb = prm.tile([P, 1 + L, H], bf16)
        nc.sync.dma_start(pk_sb[:], pk)
        hwv_sb = prm.tile([P, H], f32)
        nc.sync.dma_start(hwv_sb[:], hwv)
        if not trivial_affine:
            aux_sb = prm.tile([P, 2 * L + 2, H], f32)
            nc.sync.dma_start(aux_sb[:], aux)

        for g in [gg for _ in range(reps) for gg in range(graphs)]:
            # ---- load B^T (bf16) and x^T (bf16) ----
            bt_sb = bt_pool.tile([P, NT, N], bf16)
            nc.sync.dma_start(
                bt_sb[:], bt[g].rearrange("(jb p) i -> p jb i", p=P))
            xt_sb = xt_pool.tile([D, N], bf16)
            nc.sync.dma_start(xt_sb[:], xt[g])

            # ---- h0 = x @ proj_w (+ proj_b) ----
            h = h_pool.tile([P, NT, H], f32)
            for half in range(2):
                hp = ps_w.tile([P, HALF, H], f32)
                for q in range(HALF):
                    i = half * HALF + q
                    nc.tensor.matmul(
                        hp[:, q, :], lhsT=xt_sb[:, i * P:(i + 1) * P],
                        rhs=pk_sb[:, 0, :], start=True, stop=True)
                hs = slice(half * HALF, (half + 1) * HALF)
                if trivial_affine:
                    nc.vector.tensor_copy(h[:, hs, :], hp[:, :, :])
                else:
                    nc.vector.tensor_tensor(
                        h[:, hs, :], hp[:, :, :],
                        _bmid(aux_sb[:, 2 * L, :], HALF), op=ADD)

            # ---- layers ----
            for l in range(L):
                # LN stats: mu, rstd per node
                musum = sm_pool.tile([P, NT], f32, tag="musum")
                nc.vector.tensor_reduce(musum[:], h[:, :, :], axis=AX, op=ADD)
                sqh = big_pool.tile([P, NT, H], f32, tag="big")
                nc.scalar.activation(sqh[:], h[:, :, :], AF.Square)
                ssq = sm_pool.tile([P, NT], f32, tag="ssq")
                nc.vector.tensor_reduce(ssq[:], sqh[:, :, :], axis=AX, op=ADD)
                mu = sm_pool.tile([P, NT], f32, tag="mu")
                nc.vector.tensor_scalar(mu[:], musum[:], 1.0 / H, None, MUL)
                var = sm_pool.tile([P, NT], f32, tag="var")
                nc.vector.tensor_tensor(var[:], mu[:], mu[:], op=MUL)
                nc.vector.tensor_scalar(ssq[:], ssq[:], 1.0 / H, None, MUL)
                nc.vector.tensor_tensor(var[:], ssq[:], var[:], op=SUB)
                stdv = sm_pool.tile([P, NT], f32, tag="stdv")
                nc.scalar.activation(stdv[:], var[:], AF.Sqrt, bias=eps_sb[:, 0:1])
                rstd = sm_pool.tile([P, NT], f32, tag="rstd")
                nc.vector.reciprocal(rstd[:], stdv[:])

                # z = relu((h - mu) * rstd [* g + b])  (bf16, node-major)
                # elementwise work on the otherwise-idle GpSimd engine
                zf = big_pool.tile([P, NT, H], f32, tag="big")
                nc.gpsimd.tensor_tensor(
                    zf[:, :, :], h[:, :, :], mu[:].broadcast_to([P, NT, H]), op=SUB)
                nc.gpsimd.tensor_tensor(
                    zf[:, :, :], zf[:, :, :], rstd[:].broadcast_to([P, NT, H]), op=MUL)
                if not trivial_affine:
                    nc.vector.tensor_tensor(
                        zf[:, :, :], zf[:, :, :], _bmid(aux_sb[:, l, :], NT), op=MUL)
                    nc.vector.tensor_tensor(
                        zf[:, :, :], zf[:, :, :], _bmid(aux_sb[:, L + l, :], NT), op=ADD)
                z = z_pool.tile([P, NT, H], bf16)
                nc.scalar.activation(z[:, :, :], zf[:, :, :], AF.Relu)

                # s^T = z^T @ B^T, col-tiled 2x: PE cols 0:63 (chain A) compute
                # node-columns 0:1023 -> psum rows 0:64; cols 64:127 (chain B)
                # compute 1024:2047 -> psum rows 64:128.  Both chains share the
                # z weights and run concurrently.
                # st layout: [0:64, 0:1024] = s^T[:, 0:1024];
                #            [64:128, 0:1024] = s^T[:, 1024:2048]
                if col_tile:
                    st = st_pool.tile([P, N // 2], bf16, tag="st")
                else:
                    st = st_pool.tile([H, N], bf16, tag="st")
                if skip_agg:
                    nc.vector.memset(st[:, :], 0.0)
                elif col_tile:
                    pss = ps_s.tile([P, N // 2], f32)
                    for cp in range(2):   # chunk pair: A chunk cp, B chunk 2+cp
                        csA = slice(cp * CHW, (cp + 1) * CHW)
                        csB = slice((2 + cp) * CHW, (3 + cp) * CHW)
                        csP = slice(cp * CHW, (cp + 1) * CHW)
                        for jb in range(NT):
                            nc.tensor.matmul(
                                pss[0:H, csP], lhsT=z[:, jb, :],
                                rhs=bt_sb[:, jb, csA],
                                start=(jb == 0), stop=(jb == NT - 1))
                            nc.tensor.matmul(
                                pss[H:P, csP], lhsT=z[:, jb, :],
                                rhs=bt_sb[:, jb, csB],
                                start=(jb == 0), stop=(jb == NT - 1))
                        # copy each half-chunk out as it completes; full-width
                        # [128, 512] copies cover both chains in one op
                        if cp == 0:
                            nc.vector.tensor_copy(st[:, csP], pss[:, csP])
                        else:
                            nc.scalar.copy(st[:, csP], pss[:, csP])
                else:
                    # single-chain layout: psum [H, 1024] tiles, st [H, N]
                    for half in range(2):
                        pss = ps_s.tile([P, N // 2], f32)
                        for cp in range(2):
                            c = half * 2 + cp
                            cs = slice(c * CHW, (c + 1) * CHW)
                            csP = slice(cp * CHW, (cp + 1) * CHW)
                            for jb in range(NT):
                                nc.tensor.matmul(
                                    pss[0:H, csP], lhsT=z[:, jb, :],
                                    rhs=bt_sb[:, jb, cs],
                                    start=(jb == 0), stop=(jb == NT - 1))
                            if cp == 0:
                                nc.vector.tensor_copy(st[:, cs], pss[0:H, csP])
                            else:
                                nc.scalar.copy(st[:, cs], pss[0:H, csP])

                # h += s @ Wt[l]   (lhsT = s^T slices -> node-major psum);
                # node tiles i<8 read st rows 0:64 (PE rows 0:63), i>=8 read
                # rows 64:128 (PE rows 64:127) -> row-tiled concurrency.
                for half in range(0 if skip_w else 2):
                    wp = ps_w.tile([P, HALF, H], f32)
                    for q in range(HALF):
                        i = half * HALF + q
                        if not col_tile:
                            lhsT = st[:, i * P:(i + 1) * P]
                            rhs = pk_sb[0:H, 1 + l, :]
                        elif i < NT // 2:
                            lhsT = st[0:H, i * P:(i + 1) * P]
                            rhs = pk_sb[0:H, 1 + l, :]
                        else:
                            j = i - NT // 2
                            lhsT = st[H:P, j * P:(j + 1) * P]
                            rhs = pk_sb[H:P, 1 + l, :]
                        nc.tensor.matmul(
                            wp[:, q, :], lhsT=lhsT, rhs=rhs,
                            start=True, stop=True)
                    hs = slice(half * HALF, (half + 1) * HALF)
                    nc.vector.tensor_tensor(
                        h[:, hs, :], h[:, hs, :], wp[:, :, :], op=ADD)

            # ---- head: out = h @ head_w (+ head_b) ----
            th = big_pool.tile([P, NT, H], f32, tag="big")
            nc.gpsimd.tensor_tensor(
                th[:, :, :], h[:, :, :], _bmid(hwv_sb[:, :], NT), op=MUL)
            osb = ob_pool.tile([P, NT], f32)
            nc.vector.tensor_reduce(osb[:], th[:, :, :], axis=AX, op=ADD)
            if not trivial_affine:
                nc.vector.tensor_scalar(
                    osb[:], osb[:], aux_sb[:, 2 * L + 1, 0:1], None, ADD)
            nc.sync.dma_start(
                out=out[g].rearrange("(ib p) one -> p (ib one)", p=P),
                in_=osb[:, :])

    nc.compile()
    return nc


_NC = {}


def _get_nc(trivial_affine=True):
    key = trivial_affine
    if key not in _NC:
        _NC[key] = _build_nc(trivial_affine=trivial_affine)
    return _NC[key]


def _prep_in_maps(inputs):
    x = np.asarray(inputs["x"], np.float32)
    adj = np.asarray(inputs["adj"], np.float32)
    proj_w = np.asarray(inputs["proj_w"], np.float32)
    proj_b = np.asarray(inputs["proj_b"], np.float32)
    ln_g = np.asarray(inputs["ln_g"], np.float32)
    ln_b = np.asarray(inputs["ln_b"], np.float32)
    conv_w = np.asarray(inputs["conv_w"], np.float32)
    head_w = np.asarray(inputs["head_w"], np.float32)
    head_b = np.asarray(inputs["head_b"], np.float32)

    trivial_affine = bool(
        np.all(ln_g == 1.0) and np.all(ln_b == 0.0)
        and np.all(proj_b == 0.0) and np.all(head_b == 0.0))

    # Wt[l] = (1-beta) I + beta conv_w[l], replicated in both partition
    # halves (row-tiled w-matmuls read rows 0:64 or 64:128)
    pkh = np.zeros((P, 1 + L, H), np.float32)
    pkh[:, 0, :] = proj_w
    for l in range(L):
        beta = math.log(LAMDA / (l + 1) + 1.0)
        wt = (1.0 - beta) * np.eye(H, dtype=np.float32) + beta * conv_w[l]
        pkh[:H, 1 + l, :] = wt
        pkh[H:, 1 + l, :] = wt

    # B^T per graph, bf16: bt[j, i] = 0.9*d_j*d_i*((A!=0)^T + I)[j,i] + 0.1*I
    ey = np.eye(N, dtype=np.float32)
    bt_all = np.empty((BS, N, N), ml_dtypes.bfloat16)
    sc = np.empty((N,), np.float32)
    for gi in range(BS):
        pat = adj[gi] != 0
        deg = pat.sum(-1, dtype=np.float32) + 1.0
        np.sqrt(0.9 / deg, out=sc)                      # sqrt(0.9) * deg^-1/2
        b = pat.T.astype(np.float32)
        b += ey
        b *= sc[:, None]
        b *= sc[None, :]
        b[np.arange(N), np.arange(N)] += 0.1
        bt_all[gi] = b.astype(ml_dtypes.bfloat16)

    xt_all = np.ascontiguousarray(
        x.transpose(0, 2, 1)).astype(ml_dtypes.bfloat16)

    shared = {
        "pk": pkh.astype(ml_dtypes.bfloat16),
        "hwv": np.ascontiguousarray(
            np.broadcast_to(head_w[:, 0][None, :], (P, H))),
    }
    if not trivial_affine:
        aux = np.zeros((P, 2 * L + 2, H), np.float32)
        aux[:, :L, :] = np.broadcast_to(ln_g[:, None, :], (L, P, H)).transpose(1, 0, 2)
        aux[:, L:2 * L, :] = np.broadcast_to(ln_b[:, None, :], (L, P, H)).transpose(1, 0, 2)
        aux[:, 2 * L, :] = proj_b[None, :]
        aux[:, 2 * L + 1, 0] = head_b[0]
        shared["aux"] = aux

    in_maps = []
    for c in range(NCORES):
        sl = slice(c * G, (c + 1) * G)
        in_maps.append(dict(
            shared,
            bt=np.ascontiguousarray(bt_all[sl]),
            xt=np.ascontiguousarray(xt_all[sl]),
        ))
    return in_maps, trivial_affine


def kernel(**inputs) -> np.ndarray:
    in_maps, trivial_affine = _prep_in_maps(inputs)
    nc = _get_nc(trivial_affine)
    res = run_bass_kernel_spmd(nc, in_maps, list(range(NCORES)))
    return np.concatenate([res.results[c]["out"] for c in range(NCORES)], axis=0)
